# revision 39
# baseline (speedup 1.0000x reference)
"""Trainium2 Bass kernel for nn_CrossAttentionT2S (fused pos-embed cross-attention).

Sharding: data-parallel over the true batch axis b=8, one batch element per
NeuronCore. All tensors bf16 on device; feature-major ("transposed",
[feature, token]) layouts so matmuls contract over the partition dim.

Positional embeddings are folded into s_x / t_x on the host (cheap elementwise
prep), so the device kernel starts matmuls as soon as the x DMAs land.

Per core (NT=1568 q tokens, 1568 kv tokens, 12 heads, dh=64):
  kT = k_w @ t (feature-major, 6 chunks of 2 heads) (PE, evac on DVE + bias)
  qT = (q_w*SCALE) @ s + q_b*SCALE                  (PE, evac DVE)
  V' = token-major [128tok, parity, 6, 128]: [ones64|v]  (PE, evac DVE)
  per (qb in 512,512,512 + rump32, c6 head-pair, ki in 13):
    S[k128, q512]x2 heads — two row-tiled matmuls (0,0)/(64,0), concurrent
    P = exp(S) — ONE ScalarE activation per (c6,qb,ki), [128, 1024] free
    O~/den: AV matmul lhsT=V'[h] M=128: psum [128,512]: den on parts 0:64,
       O~ on 64:128
    evac: DVE reciprocal_approx_fast(den half) -> rcp, DVE mul -> OT bf16
  out = proj_w @ O + proj_b (PE, interleaved with attention; DVE evac, DMA out)

ScalarE runs ONLY exp (~270us); PE work (QK/AV/projections ~310us) is
interleaved to fill exp-wait gaps.
"""
import sys
import types
from contextlib import ExitStack

import numpy as np
import ml_dtypes

import concourse.bass as bass
import concourse.mybir as mybir
import concourse.tile as tile
from concourse import bacc
from concourse.bass_utils import run_bass_kernel_spmd

# ---------------------------------------------------------------- constants
DIM = 768
H = 12
DH = 64
T = 8
TS = 8
APATCH = 196
VP = 196
B = 8
NT = APATCH * TS          # 1568 tokens per core, both q and kv side
SCALE = DH ** -0.5
NCH = DIM // 128          # 6 feature chunks (2 heads each)
KT = 13                   # k tiles: 12 full 128 + rump 32
KR = NT - 12 * 128        # 32
QB = 512                  # q block
NQB = 3                   # full q blocks; rump = 32
QR = NT - NQB * QB        # 32
F32 = mybir.dt.float32
BF16 = mybir.dt.bfloat16
F8 = mybir.dt.float8e4
ADD = mybir.AluOpType.add
MULT = mybir.AluOpType.mult

_NC_CACHE = {}

import os
V_RECIP = os.environ.get("KV_RECIP", "fast")     # fast | exact
V_MEMSET = os.environ.get("KV_MEMSET", "pool")   # pool | dve
AVFP8 = os.environ.get("KV_AVFP8", "0") == "1"   # fp8 DoubleRow AV matmuls
# (measured: rel_err 1.92e-2 vs gate 2e-2 and only ~16us faster — the DR
# 256-col LDWEIGHTS can't be hidden. Kept as an option, off by default.)
NG = 6                                            # 256-token DR groups


def kw_of(ki):
    return 128 if ki < 12 else KR


def build_nc():
    nc = bacc.Bacc(None)

    # all big inputs pre-laid-out on host as [128, NCH, cols] so the DMA is
    # one fully-contiguous transfer (strided gathers run at ~84GB/s vs ~360).
    # s/t are split into a [0:512) token slice (feeds the first q/k-proj
    # blocks) and the rest, so the pipeline starts before all input landed.
    s_xa = nc.dram_tensor("s_xa", [128, NCH, QB], BF16, kind="ExternalInput")
    s_xb = nc.dram_tensor("s_xb", [128, NCH, NT - QB], BF16, kind="ExternalInput")
    t_xa = nc.dram_tensor("t_xa", [128, NCH, QB], BF16, kind="ExternalInput")
    t_xb = nc.dram_tensor("t_xb", [128, NCH, NT - QB], BF16, kind="ExternalInput")
    q_wT = nc.dram_tensor("q_wT", [128, NCH, DIM], BF16, kind="ExternalInput")
    k_wT = nc.dram_tensor("k_wT", [128, NCH, DIM], BF16, kind="ExternalInput")
    v_wT = nc.dram_tensor("v_wT", [128, NCH, DIM], BF16, kind="ExternalInput")
    proj_wT = nc.dram_tensor("proj_wT", [128, NCH, DIM], BF16, kind="ExternalInput")
    q_b2 = nc.dram_tensor("q_b2", [128, NCH], F32, kind="ExternalInput")
    k_b2 = nc.dram_tensor("k_b2", [128, NCH], F32, kind="ExternalInput")
    p_b2 = nc.dram_tensor("p_b2", [128, NCH], F32, kind="ExternalInput")
    v_br = nc.dram_tensor("v_br", [128, DIM], BF16, kind="ExternalInput")
    outT = nc.dram_tensor("outT", [DIM, NT], BF16, kind="ExternalOutput")

    with tile.TileContext(nc) as tc, ExitStack() as top:
        # ---------------- constant / persistent tiles
        cpool = top.enter_context(tc.tile_pool(name="consts", bufs=1))
        qb_t = cpool.tile([128, NCH], F32, tag="qb")
        kb_t = cpool.tile([128, NCH], F32, tag="kb")
        pb_t = cpool.tile([128, NCH], F32, tag="pb")
        vb_t = cpool.tile([128, DIM], BF16, tag="vb")

        w_pool = top.enter_context(tc.tile_pool(name="w", bufs=1))
        qw_b = w_pool.tile([128, NCH, DIM], BF16, tag="qw")
        kw_b = w_pool.tile([128, NCH, DIM], BF16, tag="kw")
        vw_b = w_pool.tile([128, NCH, DIM], BF16, tag="vw")
        pw_b = w_pool.tile([128, NCH, DIM], BF16, tag="pw")
        qw = [qw_b[:, c, :] for c in range(NCH)]
        kw = [kw_b[:, c, :] for c in range(NCH)]
        vw = [vw_b[:, c, :] for c in range(NCH)]
        pw = [pw_b[:, c, :] for c in range(NCH)]

        # x (+pos, folded on host), bf16 feature-major
        xs_pool = top.enter_context(tc.tile_pool(name="xs", bufs=1))
        sT_b = xs_pool.tile([128, NCH, NT], BF16, tag="sT")
        tT_b = xs_pool.tile([128, NCH, NT], BF16, tag="tT")
        sT = [sT_b[:, c, :] for c in range(NCH)]
        tT = [tT_b[:, c, :] for c in range(NCH)]

        # ---- input DMAs in priority waves. One dma_start only sustains
        # ~117GB/s, so each tensor is split into per-chunk transfers spread
        # over the three DMA-capable queues (sync/scalar HWDGE + gpsimd
        # SWDGE); together they reach the ~360GB/s HBM limit. Tiny DVE
        # corner copies order the waves around the exp-critical path:
        #   W1 t[0:512]+k_w -> W2 s[0:512]+q_w -> W3 v_w -> W4 t[512:]
        #   -> W5 s[512:]+proj_w
        DQ = [nc.sync, nc.scalar, nc.gpsimd]

        def spread_dma(dst, src):
            for c in range(NCH):
                DQ[c % 3].dma_start(dst[:, c, :], src[:, c, :])

        def gate(dst, dsl, src, ssl):
            # per-chunk corner dep: dst chunk c waits for src wave to land
            for c in range(NCH):
                nc.vector.tensor_copy(dst[0:1, c, dsl:dsl + 2],
                                      src[0:1, 3 + c % 3, ssl:ssl + 2])

        with tc.high_priority():
            spread_dma(tT_b[:, :, 0:QB], t_xa)
            spread_dma(kw_b, k_wT)
        nc.scalar.dma_start(kb_t[:], k_b2[:])
        nc.scalar.dma_start(qb_t[:], q_b2[:])
        nc.scalar.dma_start(pb_t[:], p_b2[:])
        nc.scalar.dma_start(vb_t[:], v_br[:])
        gate(sT_b, 0, tT_b, 0)
        gate(qw_b, 0, tT_b, 0)
        spread_dma(sT_b[:, :, 0:QB], s_xa)
        spread_dma(qw_b, q_wT)
        gate(vw_b, 0, qw_b, 0)
        spread_dma(vw_b, v_wT)
        gate(tT_b, QB, vw_b, 0)
        spread_dma(tT_b[:, :, QB:NT], t_xb)
        gate(sT_b, QB, tT_b, QB)
        gate(pw_b, 0, tT_b, QB)
        spread_dma(sT_b[:, :, QB:NT], s_xb)
        spread_dma(pw_b, proj_wT)

        # psum pools
        qk_psum = top.enter_context(tc.tile_pool(name="qkps", bufs=2, space="PSUM"))
        av_psum = top.enter_context(tc.tile_pool(name="avps", bufs=2, space="PSUM"))
        gm_psum = top.enter_context(tc.tile_pool(name="gmps", bufs=2, space="PSUM"))
        QSL = [(i * QB, QB) for i in range(NQB)] + [(NQB * QB, QR)]

        def emit_proj(ws, xsrc, dst, bias_t, c_out, q0, qn):
            """dst[c_out][:, q0:q0+qn] = ws.T @ x (+bias), bf16 evac on DVE."""
            ps = gm_psum.tile([128, 512], F32, tag="gm")
            for c in range(NCH):
                nc.tensor.matmul(
                    ps[:, 0:qn],
                    ws[c][:, c_out * 128:(c_out + 1) * 128],
                    xsrc[c][:, q0:q0 + qn],
                    start=(c == 0), stop=(c == NCH - 1),
                )
            nc.vector.tensor_scalar_add(
                dst[c_out][:, q0:q0 + qn], ps[:, 0:qn], bias_t[:, c_out:c_out + 1]
            )

        # q/k feature-major bf16; V' token-major bf16 with ones blocks
        qkT_pool = top.enter_context(tc.tile_pool(name="qkT", bufs=NCH))
        qT = [qkT_pool.tile([128, NT], BF16, tag="qT", name=f"qT{c}") for c in range(NCH)]
        kT = [qkT_pool.tile([128, NT], BF16, tag="kT", name=f"kT{c}") for c in range(NCH)]
        if AVFP8:
            # fp8 DoubleRow V': per 256-token group g, [tok, j, parity, pair,
            # 128] where token = 256g + 128j + p; head h = 2*pair+parity.
            # M layout per head: [ones 0:64 | v 64:128]. Rump 32 tokens stay
            # bf16 in vR.
            vP_pool = top.enter_context(tc.tile_pool(name="vP", bufs=NG + 1))
            vdr = [vP_pool.tile([128, 2, 2, NCH, 128], F8, tag="vdr",
                                name=f"vdr{g}") for g in range(NG)]
            vR = vP_pool.tile([128, 2, NCH, 128], BF16, tag="vR", name="vR")
            vP = None
        else:
            vP_pool = top.enter_context(tc.tile_pool(name="vP", bufs=KT))
            # layout: [tok, parity, pair, 128]; head h = 2*pair+parity
            vP = [vP_pool.tile([128, 2, NCH, 128], BF16, tag="vP", name=f"vP{k}")
                  for k in range(KT)]
        OT_pool = top.enter_context(tc.tile_pool(name="OT", bufs=NCH))
        OT = [OT_pool.tile([128, NT], BF16, tag="OT", name=f"OT{c}") for c in range(NCH)]

        ms_eng = nc.gpsimd if V_MEMSET == "pool" else nc.vector
        if AVFP8:
            for g in range(NG):
                ms_eng.memset(vdr[g][:, :, :, :, 0:64], 1.0)
            ms_eng.memset(vR[:KR, :, :, 0:64], 1.0)
        else:
            for k in range(KT):
                ms_eng.memset(vP[k][:kw_of(k), :, :, 0:64], 1.0)

        # K-proj first token block for ALL output chunks — needs only the
        # wave-1 DMAs (t[0:512] + k_w), and fills the PE until wave-2 (s/q_w)
        # lands. Later slices are paced into the ki loop via pend.
        with nc.named_scope("p0_kproj"):
            for c_out in range(NCH):
                emit_proj(kw, tT, kT, kb_t, c_out, 0, QB)

        P_pool = top.enter_context(tc.tile_pool(name="P", bufs=3))
        rcp_pool = top.enter_context(tc.tile_pool(name="rcp", bufs=1))
        Pr_pool = top.enter_context(tc.tile_pool(name="Pr", bufs=2))
        ost_pool = top.enter_context(tc.tile_pool(name="ost", bufs=2))

        def emit_vproj(k):
            """V' for k-tile k. v_wT cols pre-reordered on host:
            group0 = even heads' v dims, group1 = odd heads'."""
            kwid = kw_of(k)
            for g in range(2):
                ps = gm_psum.tile([128, 512], F32, tag="gm")
                for c in range(NCH):
                    nc.tensor.matmul(
                        ps[:kwid, 0:384],
                        tT[c][:, k * 128:k * 128 + kwid],
                        vw[c][:, g * 384:(g + 1) * 384],
                        start=(c == 0), stop=(c == NCH - 1),
                    )
                if AVFP8:
                    dst = (vdr[k // 2][:kwid, k % 2, g, :, 64:128] if k < 12
                           else vR[:kwid, g, :, 64:128])
                else:
                    dst = vP[k][:kwid, g, :, 64:128]
                src = ps[:kwid, 0:384].rearrange("p (h d) -> p h d", d=DH)
                bia = vb_t[:kwid, g * 384:(g + 1) * 384].rearrange(
                    "p (h d) -> p h d", d=DH)
                nc.vector.tensor_tensor(dst, src, bia, ADD)

        # out-proj DMA queues: NOT scalar — a ~600ns DIRECT2D submission on
        # the scalar queue delays the exp stream
        ODMA = [nc.sync, nc.gpsimd]
        odma_i = [0]

        def emit_outproj(c_out, q0, qn):
            ps = gm_psum.tile([128, 512], F32, tag="gm")
            for c in range(NCH):
                nc.tensor.matmul(
                    ps[:, 0:qn],
                    pw[c][:, c_out * 128:(c_out + 1) * 128],
                    OT[c][:, q0:q0 + qn],
                    start=(c == 0), stop=(c == NCH - 1),
                )
            oe = ost_pool.tile([128, 512], BF16, tag="ost")
            nc.vector.tensor_scalar_add(
                oe[:, 0:qn], ps[:, 0:qn], pb_t[:, c_out:c_out + 1]
            )
            eng = ODMA[odma_i[0] % len(ODMA)]
            odma_i[0] += 1
            eng.dma_start(outT[c_out * 128:(c_out + 1) * 128, q0:q0 + qn],
                          oe[:, 0:qn])

        filler = []  # deferred out-proj emissions (no forward PE deps)

        def filler_emit(n):
            for _ in range(n):
                if filler:
                    filler.pop(0)()

        def av_evac(c6, avA, avB, q0, qn):
            """Normalize + evac both heads of chunk c6 for q slice [q0, q0+qn)."""
            rcp = rcp_pool.tile([128, 1024], F32, tag="rcp")
            recip = (nc.vector.reciprocal_approx_fast if V_RECIP == "fast"
                     else nc.vector.reciprocal)
            recip(rcp[0:64, 0:qn], avA[0:64, 0:qn])
            recip(rcp[0:64, 512:512 + qn], avB[0:64, 0:qn])
            nc.vector.tensor_tensor(
                OT[c6][0:64, q0:q0 + qn], avA[64:128, 0:qn], rcp[0:64, 0:qn],
                MULT)
            nc.vector.tensor_tensor(
                OT[c6][64:128, q0:q0 + qn], avB[64:128, 0:qn],
                rcp[0:64, 512:512 + qn], MULT)

        def qk_mm(c6, ki, q0, qn):
            kwid = kw_of(ki)
            ksl = slice(ki * 128, ki * 128 + kwid)
            qk = qk_psum.tile([128, 1024], F32, tag="qk", name=f"qk{ki % 2}")
            nc.tensor.matmul(
                qk[:kwid, 0:qn], kT[c6][0:64, ksl], qT[c6][0:64, q0:q0 + qn],
                start=True, stop=True, tile_position=(0, 0),
            )
            nc.tensor.matmul(
                qk[:kwid, 512:512 + qn], kT[c6][64:128, ksl],
                qT[c6][64:128, q0:q0 + qn],
                start=True, stop=True, tile_position=(64, 0),
            )
            return qk

        def rump_qk(c6):
            """Rump q block (32 cols): batched QK psum across ki.
            (q-proj for the rump slice was filler-emitted during qbi=2.)"""
            q0 = NQB * QB
            qk = qk_psum.tile([128, 1024], F32, tag="qk", name="qkr")
            # head-major, ki padded to 16: head A fills psum bank 0,
            # head B bank 1 (concurrent pair must hit distinct banks)
            qkv = qk[:, :].rearrange("p (h k q) -> p h k q", h=2, k=16)
            for ki in range(KT):
                kwid = kw_of(ki)
                ksl = slice(ki * 128, ki * 128 + kwid)
                nc.tensor.matmul(
                    qkv[:kwid, 0, ki, :], kT[c6][0:64, ksl],
                    qT[c6][0:64, q0:q0 + QR],
                    start=True, stop=True, tile_position=(0, 0),
                )
                nc.tensor.matmul(
                    qkv[:kwid, 1, ki, :], kT[c6][64:128, ksl],
                    qT[c6][64:128, q0:q0 + QR],
                    start=True, stop=True, tile_position=(64, 0),
                )
            return qkv

        def rump_expav(c6, qkv, next_qk):
            """Exp + AV + evac for rump chunk c6; next_qk() emits the next
            chunk's QK matmuls so the PE works while ACT runs the exps."""
            q0 = NQB * QB
            avA = av_psum.tile([128, 512], F32, tag="av", name="avA")
            avB = av_psum.tile([128, 512], F32, tag="av", name="avB")
            exdt = F8 if AVFP8 else BF16
            ex = Pr_pool.tile([128, 2, 16, QR], exdt, tag="Pr")
            exk = (Pr_pool.tile([128, 2, QR], BF16, tag="Prk", name="exk")
                   if AVFP8 else None)
            for hh in range(2):
                nc.scalar.activation(
                    ex[:, hh, 0:12, :], qkv[:, hh, 0:12, :],
                    mybir.ActivationFunctionType.Exp,
                )
                nc.scalar.activation(
                    (exk[0:KR, hh, :] if AVFP8 else ex[0:KR, hh, 12, :]),
                    qkv[0:KR, hh, 12, :],
                    mybir.ActivationFunctionType.Exp,
                )
            if next_qk is not None:
                qkv2 = next_qk()
            else:
                qkv2 = None
            filler_emit(1)
            if AVFP8:
                for g in range(NG):
                    for hh in range(2):
                        nc.tensor.matmul(
                            (avA if hh == 0 else avB)[:, 0:QR],
                            vdr[g][:, :, hh, c6, :],
                            ex[:, hh, 2 * g:2 * g + 2, :],
                            start=(g == 0), stop=False,
                            perf_mode=mybir.MatmulPerfMode.DoubleRow,
                        )
                for hh in range(2):
                    nc.tensor.matmul(
                        (avA if hh == 0 else avB)[:, 0:QR],
                        vR[:KR, hh, c6, :],
                        exk[0:KR, hh, :],
                        start=False, stop=True,
                    )
            else:
                for ki in range(KT):
                    kwid = kw_of(ki)
                    for hh in range(2):
                        nc.tensor.matmul(
                            (avA if hh == 0 else avB)[:, 0:QR],
                            vP[ki][:kwid, hh, c6, :],
                            ex[:kwid, hh, ki, :],
                            start=(ki == 0), stop=(ki == KT - 1),
                        )
            av_evac(c6, avA, avB, q0, QR)
            return qkv2

        # ---------------- main attention pipeline
        pend = []
        with nc.named_scope("attn"):
            for qbi in range(NQB):
                q0 = qbi * QB
                for c6 in range(NCH):
                    # projections feeding later iterations, paced via
                    # pend (qbi==0) / filler (qbi>0) inside the ki loop
                    if qbi == 0:
                        if c6 + 1 < NCH:
                            for (a, b) in QSL[1:]:
                                pend.append(
                                    (lambda c=c6 + 1, a=a, b=b:
                                     emit_proj(kw, tT, kT, kb_t, c, a, b)))
                            pend.append(
                                (lambda c=c6 + 1:
                                 emit_proj(qw, sT, qT, qb_t, c, 0, QB)))
                        pend.append(
                            (lambda c=c6:
                             emit_proj(qw, sT, qT, qb_t, c, QB, QB)))
                    else:
                        filler.append(
                            (lambda c=c6, a=(qbi - 1) * QB:
                             emit_outproj(c, a, QB)))
                        if qbi == 1:
                            filler.append(
                                (lambda c=c6:
                                 emit_proj(qw, sT, qT, qb_t, c, 2 * QB, QB)))
                        if qbi == 2:
                            filler.append(
                                (lambda c=c6:
                                 emit_proj(qw, sT, qT, qb_t, c, NQB * QB, QR)))

                    avA = av_psum.tile([128, 512], F32, tag="av", name="avA")
                    avB = av_psum.tile([128, 512], F32, tag="av", name="avB")

                    if qbi == 0 and c6 == 0:
                        emit_proj(qw, sT, qT, qb_t, 0, 0, QB)
                    # QK pairs are emitted two-ki-at-a-time (at odd ki) so the
                    # PE sees one 64-row-mode block per TWO ki — the 64<->128
                    # tiling-mode switch costs ~105ns of array drain each.
                    qkq = {0: qk_mm(c6, 0, q0, QB)}
                    if qbi == 0 and c6 == 0:
                        # QK(1) before the vprojs: vproj waits on the wave-3
                        # v_w DMA and must not head-of-line-block the exps
                        qkq[1] = qk_mm(c6, 1, q0, QB)
                        emit_vproj(0)
                        emit_vproj(1)
                        # k-proj chunk-0 remaining slices, first in line
                        pend[0:0] = [
                            (lambda a=a, b=b:
                             emit_proj(kw, tT, kT, kb_t, 0, a, b))
                            for (a, b) in QSL[1:]]
                    else:
                        if qbi == 0:
                            if pend:
                                pend.pop(0)()
                        else:
                            # fill the ~1us QK(c6,1)-waits-exp boundary
                            filler_emit(1)
                        qkq[1] = qk_mm(c6, 1, q0, QB)
                    exd = None
                    for ki in range(KT):
                        kwid = kw_of(ki)
                        qk = qkq.pop(ki)
                        if qbi == 0:
                            if c6 == 0:
                                if ki + 2 < KT:
                                    emit_vproj(ki + 2)
                                # kQSL1/2/3 must pop before qk_mm of ki
                                # 4/8/12 (emitted at ki 3/7/11)
                                if pend and ki in (2, 4, 6, 7, 8, 9, 10, 11, 12):
                                    pend.pop(0)()
                            elif pend and (ki % 2 == 1 or ki == 12):
                                pend.pop(0)()
                        elif filler and (ki % 4 == 3):
                            filler_emit(1)
                        if not AVFP8:
                            ex = P_pool.tile([128, 2, 512], BF16, tag="P")
                            nc.scalar.activation(
                                ex[:kwid, :, :],
                                qk[:kwid, :].rearrange("p (h q) -> p h q", h=2),
                                mybir.ActivationFunctionType.Exp,
                            )
                            if ki % 2 == 1:
                                for kn in (ki + 1, ki + 2):
                                    if kn < KT:
                                        qkq[kn] = qk_mm(c6, kn, q0, QB)
                            for hh in range(2):
                                nc.tensor.matmul(
                                    (avA if hh == 0 else avB)[:, 0:QB],
                                    vP[ki][:kwid, hh, c6, :],
                                    ex[:kwid, hh, :],
                                    start=(ki == 0), stop=(ki == KT - 1),
                                )
                        elif ki == 12:
                            exr = P_pool.tile([128, 2, 512], BF16, tag="Pk")
                            nc.scalar.activation(
                                exr[:kwid, :, :],
                                qk[:kwid, :].rearrange("p (h q) -> p h q", h=2),
                                mybir.ActivationFunctionType.Exp,
                            )
                            for hh in range(2):
                                nc.tensor.matmul(
                                    (avA if hh == 0 else avB)[:, 0:QB],
                                    vR[:kwid, hh, c6, :],
                                    exr[:kwid, hh, :],
                                    start=False, stop=True,
                                )
                        else:
                            if ki % 2 == 0:
                                exd = P_pool.tile([128, 2, 2, 512], F8, tag="P")
                            nc.scalar.activation(
                                exd[:, ki % 2, :, :],
                                qk[:, :].rearrange("p (h q) -> p h q", h=2),
                                mybir.ActivationFunctionType.Exp,
                            )
                            if ki % 2 == 1:
                                for kn in (ki + 1, ki + 2):
                                    if kn < KT:
                                        qkq[kn] = qk_mm(c6, kn, q0, QB)
                                g = ki // 2
                                for hh in range(2):
                                    nc.tensor.matmul(
                                        (avA if hh == 0 else avB)[:, 0:QB],
                                        vdr[g][:, :, hh, c6, :],
                                        exd[:, :, hh, :],
                                        start=(g == 0), stop=False,
                                        perf_mode=mybir.MatmulPerfMode.DoubleRow,
                                    )
                    av_evac(c6, avA, avB, q0, QB)

            # ---- rump q block (32 cols): overlaps the final qb2 out-projs
            with nc.named_scope("rump"):
                qkv = rump_qk(0)
                for c6 in range(NCH):
                    nxt = (lambda c=c6 + 1: rump_qk(c)) if c6 + 1 < NCH else None
                    qkv = rump_expav(c6, qkv, nxt)
                    filler.append(
                        (lambda c=c6: emit_outproj(c, (NQB - 1) * QB, QB)))

            # ---- drain remaining filler + final out-proj slices
            with nc.named_scope("tail"):
                filler_emit(len(filler))
                for c6 in range(NCH):
                    emit_outproj(c6, NQB * QB, QR)

    nc.finalize()
    return nc


def _install_axon_ntff_shim():
    if "antenv.axon_hooks" in sys.modules:
        return
    mod = types.ModuleType("antenv.axon_hooks")
    mod._hook = None
    mod.set_axon_ntff_profile_hook = lambda h: setattr(mod, "_hook", h)
    mod.get_axon_ntff_profile_hook = lambda: mod._hook
    sys.modules["antenv.axon_hooks"] = mod
    try:
        import antenv

        antenv.axon_hooks = mod
        from trn_agent_boot.trn_boot import _ntff_profile_via_ctypes

        hook = _ntff_profile_via_ctypes("/opt/axon/libaxon_pjrt.so")
        if hook is not None:
            mod.set_axon_ntff_profile_hook(hook)
    except Exception:
        pass


def prep_inputs(s_x, t_x, clip_space_pos, vmae_space_pos, clip_temporal_pos,
                vmae_temporal_pos, q_w, q_b, kv_w, kv_b, proj_w, proj_b):
    """Host-side sharding/layout prep. Returns list of 8 per-core input maps."""
    f = np.float32
    bf = ml_dtypes.bfloat16

    def sb_layout(a):
        # [DIM, cols] -> [128, NCH, cols] matching the on-chip SBUF layout
        return np.ascontiguousarray(
            np.asarray(a).reshape(NCH, 128, -1).transpose(1, 0, 2)).astype(bf)

    q_wT = sb_layout(np.asarray(q_w).T * SCALE)
    k_wT = sb_layout(np.asarray(kv_w)[:DIM].T)
    # v weight rows reordered: [even heads' v dims | odd heads' v dims]
    v_w = np.asarray(kv_w)[DIM:]         # [768 out, 768 in]
    v_b = np.asarray(kv_b)[DIM:]
    order = np.concatenate([
        np.arange(DIM).reshape(H, DH)[0::2].reshape(-1),
        np.arange(DIM).reshape(H, DH)[1::2].reshape(-1),
    ])
    v_wT = sb_layout(v_w[order].T)
    v_br = np.ascontiguousarray(
        np.broadcast_to(v_b[order].reshape(1, DIM), (128, DIM))).astype(bf)
    proj_wT = sb_layout(np.asarray(proj_w).T)
    q_b2 = np.ascontiguousarray(
        (np.asarray(q_b) * SCALE).reshape(NCH, 128).T, dtype=f)
    k_b2 = np.ascontiguousarray(
        np.asarray(kv_b)[:DIM].reshape(NCH, 128).T, dtype=f)
    p_b2 = np.ascontiguousarray(np.asarray(proj_b).reshape(NCH, 128).T, dtype=f)

    # fold positional embeddings into x on host (elementwise, cheap)
    sp_s = np.asarray(clip_space_pos, dtype=f)[:, None, :]    # (AP, 1, D)
    tp_s = np.asarray(clip_temporal_pos, dtype=f)[None, :, :]  # (1, TS, D)
    sp_t = np.asarray(vmae_space_pos, dtype=f)[:, None, :]    # (VP, 1, D)
    tp_t = np.asarray(vmae_temporal_pos, dtype=f)[None, :, :]  # (1, T, D)

    in_maps = []
    for b in range(B):
        s_slice = np.asarray(s_x, dtype=f)[:, b * TS:(b + 1) * TS, :] + sp_s + tp_s
        t_slice = np.asarray(t_x, dtype=f)[1:, b * T:(b + 1) * T, :] + sp_t + tp_t
        s_xT = sb_layout(s_slice.transpose(2, 0, 1).reshape(DIM, NT))
        t_xT = sb_layout(t_slice.transpose(2, 0, 1).reshape(DIM, NT))
        in_maps.append({
            "s_xa": np.ascontiguousarray(s_xT[:, :, 0:QB]),
            "s_xb": np.ascontiguousarray(s_xT[:, :, QB:NT]),
            "t_xa": np.ascontiguousarray(t_xT[:, :, 0:QB]),
            "t_xb": np.ascontiguousarray(t_xT[:, :, QB:NT]),
            "q_wT": q_wT, "k_wT": k_wT, "v_wT": v_wT, "proj_wT": proj_wT,
            "q_b2": q_b2, "k_b2": k_b2, "p_b2": p_b2, "v_br": v_br,
        })
    return in_maps


def unshard_output(results):
    """results: list of 8 dicts with 'outT' [768, 1568] -> (196, 64, 768)."""
    out = np.empty((APATCH, B * TS, DIM), dtype=np.float32)
    for b in range(B):
        o = results[b]["outT"].astype(np.float32).reshape(DIM, APATCH, TS)
        out[:, b * TS:(b + 1) * TS, :] = o.transpose(1, 2, 0)
    return out


def kernel(**inputs):
    _install_axon_ntff_shim()
    in_maps = prep_inputs(**inputs)
    if "nc" not in _NC_CACHE:
        _NC_CACHE["nc"] = build_nc()
    nc = _NC_CACHE["nc"]
    res = run_bass_kernel_spmd(nc, in_maps, core_ids=list(range(B)))
    return unshard_output(res.results)


if __name__ == "__main__":
    rng = np.random.default_rng(0)
    fake = {
        "s_x": rng.standard_normal((APATCH, B * TS, DIM), dtype=np.float32),
        "t_x": rng.standard_normal((VP + 1, B * T, DIM), dtype=np.float32),
        "clip_space_pos": SCALE * rng.standard_normal((APATCH, DIM), dtype=np.float32),
        "vmae_space_pos": SCALE * rng.standard_normal((VP, DIM), dtype=np.float32),
        "clip_temporal_pos": SCALE * rng.standard_normal((TS, DIM), dtype=np.float32),
        "vmae_temporal_pos": SCALE * rng.standard_normal((T, DIM), dtype=np.float32),
        "q_w": (0.02 * rng.standard_normal((DIM, DIM))).astype(np.float32),
        "q_b": np.zeros(DIM, np.float32),
        "kv_w": (0.02 * rng.standard_normal((2 * DIM, DIM))).astype(np.float32),
        "kv_b": np.zeros(2 * DIM, np.float32),
        "proj_w": (0.02 * rng.standard_normal((DIM, DIM))).astype(np.float32),
        "proj_b": np.zeros(DIM, np.float32),
    }
    out = kernel(**fake)
    print("out", out.shape, out.dtype)
